# revision 11
# baseline (speedup 1.0000x reference)
"""Causal self-attention kernel for Trainium2, 8-core SPMD.

Problem: x[4,2048,1024], W_qkv[1024,3072], b_qkv[3072], W_proj[1024,1024],
b_proj[1024]; 16 heads, head_dim 64, causal softmax attention.

Sharding: 8 cores = 4 batches x 2 head-groups (8 heads each). Each core
computes its batch's attention for its 8 heads plus the partial output
projection over its 512 input dims; the host sums the two partial
projections per batch and adds the biases that commute with attention
(b_proj, and b_v @ W_proj since softmax rows sum to 1).

On-device dataflow per core (matmul: out = lhsT.T @ rhs, contraction on the
partition dim; PE cost ~= N_out_cols regardless of K/M, so every stage is
arranged to minimize total streamed columns):
  A. V = x @ Wv (bf16) per t-chunk; QK^T = (x @ Wqk)^T for chunks 0,4.
  B. Per head-pair p (heads 2p at partitions 0-63, 2p+1 at 64-127), k-tiles
     kt processed DESCENDING 15..0: S^T[kt,q] = K-tile^T @ Q over the exact
     causal span q in [128*kt, T); P^T = exp(S^T/8) (ACT, psum->bf16 sbuf
     tiles trimmed to the causal span); diagonal 128-block masked by 0/1
     mult. Remaining QK^T chunks are interleaved into the score streams.
  C. att@V flipped: O[q,128d] accumulates per q-tile qt DESCENDING;
     chain qt = sum over kt<=qt of P^T[kt, qt-chunk].T @ (V|ones)[kt] with
     N=65 (col 64 = softmax rowsum). Chains of pair p-1 interleave into
     pair p's score stream; chain qt's first matmul reads P^T tile kt=qt,
     freeing that slot for pair p's same-kt exp in the same step
     (descending order makes frees match the next pair's allocations).
  D. normalize per (head, qt): recip = 1/rowsum [128,1]; per-partition
     tensor_scalar mult -> o_pair bf16. Transpose [128q,128d] -> [128d,128q]
     via XBAR DMA-transpose (DMA engines, off the PE) into O^T[din, t].
  E. y[t-tile] += O^T-chunk.T @ Wp-chunk; pair-3 chains, transposes and
     y-tiles interleave so the tail stays PE-dense.
"""
import contextlib

import numpy as np
import ml_dtypes

import concourse.bass as bass
import concourse.tile as tile
from concourse import bacc, mybir
from concourse.bass_utils import run_bass_kernel_spmd

F32 = mybir.dt.float32
BF16 = mybir.dt.bfloat16

B, T, D = 4, 2048, 1024
H, HD = 16, 64
NH = 8                # heads per core
DQK = 2 * NH * HD     # 1024 q+k dims per core
DV = NH * HD          # 512 v dims per core
KT = T // 128         # 16 k/q tiles of 128
SCALE = 1.0 / float(np.sqrt(HD))


def build_nc(reps=1, n_cores=8):
    nc = bacc.Bacc("TRN2", target_bir_lowering=False, debug=False,
                   enable_asserts=False, num_devices=n_cores)
    xT_d = nc.dram_tensor("xt", [D, T], BF16, kind="ExternalInput").ap()
    wqk_d = nc.dram_tensor("wqk", [D, DQK], BF16, kind="ExternalInput").ap()
    wv_d = nc.dram_tensor("wv", [D, DV], BF16, kind="ExternalInput").ap()
    bqk_d = nc.dram_tensor("bqk", [DQK], F32, kind="ExternalInput").ap()
    wp_d = nc.dram_tensor("wp", [DV, D], BF16, kind="ExternalInput").ap()
    mask_d = nc.dram_tensor("mask", [128, 128], BF16, kind="ExternalInput").ap()
    y_d = nc.dram_tensor("y", [T, D], F32, kind="ExternalOutput").ap()

    xT_t = xT_d.rearrange("(ko ki) t -> ki ko t", ki=128)       # [128, 8, T]
    wqk_t = wqk_d.rearrange("(ko ki) d -> ki ko d", ki=128)     # [128, 8, DQK]
    wv_t = wv_d.rearrange("(ko ki) d -> ki ko d", ki=128)       # [128, 8, DV]
    bqk_t = bqk_d.rearrange("(dc ki) -> ki dc", ki=128)         # [128, 8]
    wp_t = wp_d.rearrange("(co ci) d -> ci co d", ci=128)       # [128, 4, D]
    y_t = y_d.rearrange("(tt ti) d -> ti tt d", ti=128)         # [128, 16, D]

    with tile.TileContext(nc) as tc, contextlib.ExitStack() as ctx:
        acc = ctx.enter_context(tc.tile_pool(name="acc", bufs=1))
        cpool = ctx.enter_context(tc.tile_pool(name="cpool", bufs=1))
        ps_s = ctx.enter_context(tc.tile_pool(name="ps_s", bufs=2, space="PSUM"))
        ps_o = ctx.enter_context(tc.tile_pool(name="ps_o", bufs=2, space="PSUM"))
        ps_q = ctx.enter_context(tc.tile_pool(name="ps_q", bufs=1, space="PSUM"))

        # constants go via the gpsimd (SWDGE) queue so they don't delay the
        # first xt/wv pieces on the sync queue
        bqk_s = cpool.tile([128, 8], F32)
        nc.gpsimd.dma_start(bqk_s[:], bqk_t)
        wp_s = cpool.tile([128, 4, D], BF16)
        nc.gpsimd.dma_start(wp_s[:], wp_t)
        mask_s = cpool.tile([128, 128], BF16)
        nc.gpsimd.dma_start(mask_s[:], mask_d)

        for _ in range(reps):
            qk_sb = acc.tile([128, 8, T], BF16, tag="qk")      # QK^T [d, t]
            v_sb = acc.tile([128, KT, NH, 65], BF16, tag="v")  # V [t, h, d|1]
            ot_sb = acc.tile([128, 4, T], BF16, tag="ot")      # O^T [din, t]
            nc.vector.memset(v_sb[:, :, :, 64], 1.0)

            cd_stack = contextlib.ExitStack()
            ptp = cd_stack.enter_context(tc.tile_pool(name="ptp", bufs=1))
            opool = cd_stack.enter_context(tc.tile_pool(name="opool", bufs=2))
            tmp = cd_stack.enter_context(tc.tile_pool(name="tmp", bufs=4))
            ypool = cd_stack.enter_context(tc.tile_pool(name="ypool", bufs=2))

            ab_stack = contextlib.ExitStack()
            wqkp = ab_stack.enter_context(tc.tile_pool(name="wqkp", bufs=3))
            xpool = ab_stack.enter_context(tc.tile_pool(name="xpool", bufs=1))
            a_stack = contextlib.ExitStack()
            wvp = a_stack.enter_context(tc.tile_pool(name="wvp", bufs=1))
            wv_s = wvp.tile([128, 8, DV], BF16, tag="wv")

            # ---- A: xT resident + V-proj per t-chunk ----
            xts = []
            for tcx in range(4):
                xt = xpool.tile([128, 8, 512], BF16, tag=f"xt{tcx}",
                                name=f"xt{tcx}")
                xts.append(xt)
                for k2 in range(4):
                    if tcx == 0:
                        nc.sync.dma_start(wv_s[:, 2 * k2, :], wv_t[:, 2 * k2, :])
                    nc.sync.dma_start(xt[:, 2 * k2:2 * k2 + 2, :],
                                      xT_t[:, 2 * k2:2 * k2 + 2, bass.ts(tcx, 512)])
                    if tcx == 0:
                        nc.sync.dma_start(wv_s[:, 2 * k2 + 1, :],
                                          wv_t[:, 2 * k2 + 1, :])
                for tt in range(4):
                    pv = ps_o.tile([128, 512], F32, tag="po")
                    for k in range(8):
                        nc.tensor.matmul(pv[:], xt[:, k, bass.ts(tt, 128)],
                                         wv_s[:, k, :],
                                         start=(k == 0), stop=(k == 7))
                    nc.vector.tensor_copy(
                        v_sb[:, tcx * 4 + tt, :, 0:64],
                        pv[:].rearrange("p (h d) -> p h d", h=NH))
            a_stack.close()

            # ---- B: QK-proj. Chunk dc holds Q (dc<4) or K (dc>=4) dims for
            # pair dc%4. Chunks 0,4 run up front; the rest interleave into
            # the pair score streams as generator pieces.
            def qk_chunk_pieces(dc):
                wqk_c = wqkp.tile([128, 8, 128], BF16, tag="wqkc")
                nc.sync.dma_start(wqk_c[:], wqk_t[:, :, bass.ts(dc, 128)])
                for th in range(2):  # t halves
                    pq = ps_q.tile([128, 1024], F32, tag="sq", name=f"pq{dc}_{th}")
                    for k2 in range(4):
                        for k in (2 * k2, 2 * k2 + 1):
                            for tcx in (2 * th, 2 * th + 1):
                                nc.tensor.matmul(
                                    pq[:, bass.ts(tcx % 2, 512)],
                                    wqk_c[:, k, :], xts[tcx][:, k, :],
                                    start=(k == 0), stop=(k == 7))
                        yield
                    for tcx in (2 * th, 2 * th + 1):
                        nc.vector.tensor_scalar_add(
                            qk_sb[:, dc, bass.ts(tcx, 512)],
                            pq[:, bass.ts(tcx % 2, 512)],
                            bqk_s[:, dc:dc + 1])
                    yield

            def run_gen(g):
                if g is not None:
                    next(g, None)

            for piece in qk_chunk_pieces(0):
                pass
            for piece in qk_chunk_pieces(4):
                pass

            # ---- C/D: attention ----
            def scores_step(p, kt, pts):
                """S^T and P^T for pair p, k-tile kt, exact causal span."""
                span = T - 128 * kt
                for hp in range(2):
                    pl = slice(hp * 64, (hp + 1) * 64)
                    pt = ptp.tile([128, span], BF16, tag=f"pt{hp}_{kt}",
                                  name=f"pt{hp}_{kt}_{p}")
                    pts[hp][kt] = pt
                    for b0 in range(0, span, 1024):
                        bw = min(1024, span - b0)
                        ps = ps_s.tile([128, 1024], F32, tag="s",
                                       name=f"ps{hp}_{kt}_{b0}_{p}")
                        for off in range(0, bw, 512):
                            w = min(512, bw - off)
                            nc.tensor.matmul(
                                ps[:, bass.ds(off, w)],
                                qk_sb[pl, 4 + p, bass.ts(kt, 128)],
                                qk_sb[pl, p, bass.ds(128 * kt + b0 + off, w)],
                                start=True, stop=True)
                        nc.scalar.activation(
                            pt[:, bass.ds(b0, bw)], ps[:, bass.ds(0, bw)],
                            mybir.ActivationFunctionType.Exp, scale=SCALE)
                        if b0 == 0:
                            # mask the diagonal block (cols 0:128 of the tile)
                            nc.vector.tensor_tensor(
                                pt[:, 0:128], pt[:, 0:128], mask_s[:],
                                mybir.AluOpType.mult)

            def chain_step(p, qt, pts, op_t):
                """att@V chains for pair p, q-tile qt (+norm, +transpose)."""
                for hp in range(2):
                    po = ps_o.tile([128, 512], F32, tag="po",
                                   name=f"po{hp}_{qt}_{p}")
                    for i, kt in enumerate(range(qt, -1, -1)):
                        nc.tensor.matmul(
                            po[:, 0:65],
                            pts[hp][kt][:, bass.ts(qt - kt, 128)],
                            v_sb[:, kt, 2 * p + hp, :],
                            start=(i == 0), stop=(kt == 0))
                    recip = tmp.tile([128, 1], F32, tag="recip")
                    nc.vector.reciprocal(recip[:], po[:, 64:65])
                    nc.vector.tensor_scalar_mul(
                        op_t[:, qt, bass.ds(hp * 64, 64)],
                        po[:, 0:64], recip[:])
                nc.sync.dma_start(ot_sb[:, p, bass.ts(qt, 128)],
                                  op_t[:, qt, :], transpose=True)

            def yproj(qt):
                y_sb = ypool.tile([128, D], F32, tag="y")
                py = ps_s.tile([128, 1024], F32, tag="s", name=f"py{qt}")
                for c in range(4):
                    for do2 in range(2):
                        nc.tensor.matmul(py[:, bass.ts(do2, 512)],
                                         ot_sb[:, c, bass.ts(qt, 128)],
                                         wp_s[:, c, bass.ts(do2, 512)],
                                         start=(c == 0), stop=(c == 3))
                nc.vector.tensor_copy(y_sb[:], py[:])
                nc.sync.dma_start(y_t[:, qt, :], y_sb[:])

            fillers = {0: [qk_chunk_pieces(1), qk_chunk_pieces(5)],
                       1: [qk_chunk_pieces(2), qk_chunk_pieces(6)],
                       2: [qk_chunk_pieces(3), qk_chunk_pieces(7)],
                       3: []}
            prev = None
            for p in range(4):
                pts = {0: {}, 1: {}}
                op_t = opool.tile([128, KT, 128], BF16, tag="opair",
                                  name=f"opair{p}")
                for kt in range(KT - 1, -1, -1):
                    if prev is not None:
                        chain_step(p - 1, kt, prev[0], prev[1])
                    scores_step(p, kt, pts)
                    for g in fillers[p]:
                        run_gen(g)
                prev = (pts, op_t)
                if p == 2:
                    ab_stack.close()

            # ---- E: pair-3 chains + output projection, interleaved ----
            for qt in range(KT - 1, -1, -1):
                chain_step(3, qt, prev[0], prev[1])
                yproj(qt)
            cd_stack.close()

    nc.compile()
    return nc


_NC_CACHE = {}


def _get_nc(reps=1):
    if reps not in _NC_CACHE:
        _NC_CACHE[reps] = build_nc(reps=reps)
    return _NC_CACHE[reps]


def make_in_maps(x, W_qkv, b_qkv, W_proj):
    """Per-core input dicts. Core c: batch c//2, head-group c%2."""
    ki = np.arange(128)[:, None]
    qi = np.arange(128)[None, :]
    mask = (qi >= ki).astype(ml_dtypes.bfloat16)
    in_maps = []
    for c in range(8):
        b, hg = divmod(c, 2)
        sl_q = slice(hg * 512, (hg + 1) * 512)
        sl_k = slice(D + hg * 512, D + (hg + 1) * 512)
        sl_v = slice(2 * D + hg * 512, 2 * D + (hg + 1) * 512)
        wqk = np.concatenate([W_qkv[:, sl_q], W_qkv[:, sl_k]], axis=1)
        bqk = np.concatenate([b_qkv[sl_q], b_qkv[sl_k]])
        in_maps.append({
            "xt": np.ascontiguousarray(x[b].T).astype(ml_dtypes.bfloat16),
            "wqk": np.ascontiguousarray(wqk).astype(ml_dtypes.bfloat16),
            "wv": np.ascontiguousarray(W_qkv[:, sl_v]).astype(ml_dtypes.bfloat16),
            "bqk": np.ascontiguousarray(bqk),
            "wp": W_proj[hg * 512:(hg + 1) * 512, :].astype(ml_dtypes.bfloat16),
            "mask": mask,
        })
    return in_maps


def assemble_output(results, b_qkv, W_proj, b_proj):
    bias = b_proj + b_qkv[2 * D:] @ W_proj
    y = np.empty((B, T, D), np.float32)
    for b in range(B):
        y[b] = results[2 * b]["y"] + results[2 * b + 1]["y"] + bias
    return y


def kernel(x, W_qkv, b_qkv, W_proj, b_proj):
    x = np.asarray(x, np.float32)
    W_qkv = np.asarray(W_qkv, np.float32)
    b_qkv = np.asarray(b_qkv, np.float32)
    W_proj = np.asarray(W_proj, np.float32)
    b_proj = np.asarray(b_proj, np.float32)
    nc = _get_nc(reps=1)
    in_maps = make_in_maps(x, W_qkv, b_qkv, W_proj)
    res = run_bass_kernel_spmd(nc, in_maps, core_ids=list(range(8)))
    return assemble_output(res.results, b_qkv, W_proj, b_proj)
